# revision 21
# baseline (speedup 1.0000x reference)
"""Causal self-attention on 8 trn2 NeuronCores.

Sharding: core c = (batch b = c//2, head-group g = c%2). Each core computes
QKV projection for its 8 heads of its batch, causal flash-attention in a
transposed (S^T) layout, and a partial out-projection (its 512 rows of
w_out). Host sums the two partials per batch and adds b_out.

v2 design (from HW microbenchmarks):
- bf16 operands for projections / QK^T / out-projection (same PE rate as
  f32r but half the DMA, half the LDWEIGHTS time, half the DVE cost on
  PSUM->SBUF moves). PV stays f32r: its M=65 output (64 dims + ones column
  for the softmax denominator) is rejected by codegen for 16-bit dtypes.
- The causal mask runs on the PE: for each diagonal k-tile the QK^T matmul
  is column-restricted to q >= tile base, and one extra N=128 matmul
  accumulates -1e9 over the triangular block (ident x trin), so exp gives
  exact zeros and the DVE mask multiplies are gone.
- Normalization: reciprocal of the denominator row straight from PSUM,
  PE-broadcast via a ones[1,64] stationary, one SBUF bounce, two muls.
- Scheduler: attention is emitted grp-by-grp (grp = head-pair x 2 k-tiles);
  after each grp, filler units (next chunk's projection, previous chunks'
  out-projection) are emitted to fill the PE while the scalar engine's exp
  (the per-grp bottleneck: ~2.3us vs ~1.3us PE) catches up. The PE never
  idles a HAM window, so the clock stays at 2.4 GHz. Chunk 3 keeps its own
  k/v projection and two chunks' out-projections as late filler.
"""

import numpy as np

B = 4
T = 2048
C = 1024
HG = 512          # head channels per core (8 heads x 64)
Dh = 64
NHL = 8           # local heads per core
TCH = 512         # T-chunk (q-chunk) width
NTC = T // TCH    # 4
NCC = C // 128    # 8 contraction chunks for projections
NMT = HG // 128   # 4 row-tiles of q/k channels
VW = NHL * (Dh + 1)   # 520: V tiles with a ones column per head

_CACHE = {}


def _build_nc():
    import concourse.bass as bass
    import concourse.bacc as bacc
    import concourse.tile as tile
    import concourse.mybir as mybir
    from collections import deque

    f32 = mybir.dt.float32
    f32r = mybir.dt.float32r
    bf16 = mybir.dt.bfloat16
    AF = mybir.ActivationFunctionType

    nc = bacc.Bacc("TRN2", target_bir_lowering=False, debug=False,
                   enable_asserts=False)
    x_d = nc.dram_tensor("x", [C, T], bf16, kind="ExternalInput").ap()
    wq_d = nc.dram_tensor("wq", [C, HG], bf16, kind="ExternalInput").ap()
    wk_d = nc.dram_tensor("wk", [C, HG], bf16, kind="ExternalInput").ap()
    wv_d = nc.dram_tensor("wv", [C, HG], bf16, kind="ExternalInput").ap()
    bq_d = nc.dram_tensor("bq", [HG], f32, kind="ExternalInput").ap()
    bk_d = nc.dram_tensor("bk", [HG], f32, kind="ExternalInput").ap()
    bv_d = nc.dram_tensor("bv", [HG], f32, kind="ExternalInput").ap()
    wo_d = nc.dram_tensor("wo", [HG, C], bf16, kind="ExternalInput").ap()
    id_d = nc.dram_tensor("ident", [128, 128], bf16, kind="ExternalInput").ap()
    tri_d = nc.dram_tensor("trin", [128, 128], bf16, kind="ExternalInput").ap()
    on_d = nc.dram_tensor("ones", [1, 64], f32r, kind="ExternalInput").ap()
    vo_d = nc.dram_tensor("vones", [128, NHL], f32r, kind="ExternalInput").ap()
    y_d = nc.dram_tensor("y", [T, C], bf16, kind="ExternalOutput").ap()

    def mm(out, lhsT, rhs, start, stop, **kw):
        nc.tensor.matmul(out, lhsT, rhs, start=start, stop=stop, **kw)

    with tile.TileContext(nc) as tc:
        with tc.tile_pool(name="wp", bufs=1) as wp, \
             tc.tile_pool(name="ktp", bufs=1) as ktp, \
             tc.tile_pool(name="vp", bufs=1) as vp, \
             tc.tile_pool(name="qtp", bufs=2) as qtp, \
             tc.tile_pool(name="xtp", bufs=2) as xtp, \
             tc.tile_pool(name="esp", bufs=6) as esp, \
             tc.tile_pool(name="rp", bufs=1) as rp, \
             tc.tile_pool(name="otp", bufs=3) as otp, \
             tc.tile_pool(name="yst", bufs=2) as yst, \
             tc.tile_pool(name="psM", bufs=2, space="PSUM") as psM, \
             tc.tile_pool(name="psS", bufs=2, space="PSUM") as psS, \
             tc.tile_pool(name="psO", bufs=2, space="PSUM") as psO:

            # ---- constants (sync/HWDGE: needed first) ----
            ident = wp.tile([128, 128], bf16, tag="ident")
            nc.sync.dma_start(out=ident, in_=id_d)
            trin = wp.tile([128, 128], bf16, tag="trin")
            nc.sync.dma_start(out=trin, in_=tri_d)
            ones = wp.tile([1, 64], f32r, tag="ones")
            nc.sync.dma_start(out=ones, in_=on_d)
            # (chunk-0 x^T tiles are loaded by the first ldxt units below)

            # ---- weights etc on the gpsimd/SWDGE queues so the x-chunk
            # loads (sync/HWDGE) aren't queued behind the weights ----
            wq_sb = [wp.tile([128, HG], bf16, name=f"wq{c}", tag=f"wq{c}")
                     for c in range(NCC)]
            wk_sb = [wp.tile([128, HG], bf16, name=f"wk{c}", tag=f"wk{c}")
                     for c in range(NCC)]
            wv_sb = [wp.tile([128, HG], bf16, name=f"wv{c}", tag=f"wv{c}")
                     for c in range(NCC)]
            for c in range(NCC):
                nc.gpsimd.dma_start(out=wq_sb[c], in_=wq_d[c * 128:(c + 1) * 128, :])
            for c in range(NCC):
                nc.gpsimd.dma_start(out=wk_sb[c], in_=wk_d[c * 128:(c + 1) * 128, :])
            for c in range(NCC):
                nc.gpsimd.dma_start(out=wv_sb[c], in_=wv_d[c * 128:(c + 1) * 128, :])
            wo_sb = [wp.tile([128, C], bf16, name=f"wo{m}", tag=f"wo{m}")
                     for m in range(NMT)]
            bq_sb = [wp.tile([128, 1], f32, name=f"bq{m}", tag=f"bq{m}")
                     for m in range(NMT)]
            bk_sb = [wp.tile([128, 1], f32, name=f"bk{m}", tag=f"bk{m}")
                     for m in range(NMT)]
            for m in range(NMT):
                nc.gpsimd.dma_start(
                    out=bq_sb[m],
                    in_=bq_d[m * 128:(m + 1) * 128].rearrange("(p o) -> p o", o=1))
                nc.gpsimd.dma_start(
                    out=bk_sb[m],
                    in_=bk_d[m * 128:(m + 1) * 128].rearrange("(p o) -> p o", o=1))
            bv_bc = wp.tile([128, HG], f32, tag="bvbc")
            bv_src = bass.AP(tensor=bv_d.tensor, offset=bv_d.offset,
                             ap=[[0, 128]] + list(bv_d.ap))
            nc.gpsimd.dma_start(out=bv_bc, in_=bv_src)
            for m in range(NMT):
                nc.gpsimd.dma_start(out=wo_sb[m], in_=wo_d[m * 128:(m + 1) * 128, :])

            kt_sb = [ktp.tile([128, T], bf16, name=f"kt{m}", tag=f"kt{m}")
                     for m in range(NMT)]
            v_sb = [vp.tile([128, VW], f32r, name=f"v{t}", tag=f"v{t}")
                    for t in range(T // 128)]
            for t in range(T // 128):
                nc.gpsimd.dma_start(
                    out=v_sb[t].rearrange("p (h e) -> p h e", h=NHL)[:, :, Dh:Dh + 1],
                    in_=vo_d.rearrange("p (h e) -> p h e", e=1))

            # ======== unit closures ========
            store = {}
            ot_store = {}
            def psm_tile(shape, dtype, name):
                return psM.tile(shape, dtype, tag="mm", name=name)

            def u_ldxt(ti, c):
                def f():
                    t0 = ti * TCH
                    if ti not in store:
                        store[ti] = {"xt": [None] * NCC, "qt": [None] * NMT}
                    st = store[ti]
                    xt = xtp.tile([128, TCH], bf16, name=f"xt{c}", tag=f"xt{c}")
                    st["xt"][c] = xt
                    nc.sync.dma_start(
                        out=xt, in_=x_d[c * 128:(c + 1) * 128, t0:t0 + TCH])
                return f, 0.0

            def u_pq(ti, m):
                def f():
                    st = store[ti]
                    pq = psm_tile([128, TCH], f32, "pq")
                    for c in range(NCC):
                        mm(pq, wq_sb[c][:, m * 128:(m + 1) * 128],
                           st["xt"][c], c == 0, c == NCC - 1)
                    qtm = qtp.tile([128, TCH], bf16, name=f"qt{m}", tag=f"qt{m}")
                    st["qt"][m] = qtm
                    nc.vector.tensor_scalar_add(qtm, pq, bq_sb[m])
                return f, 1750.0

            def u_pk(ti, m):
                def f():
                    st = store[ti]
                    t0 = ti * TCH
                    pk = psm_tile([128, TCH], f32, "pk")
                    for c in range(NCC):
                        mm(pk, wk_sb[c][:, m * 128:(m + 1) * 128],
                           st["xt"][c], c == 0, c == NCC - 1)
                    nc.vector.tensor_scalar_add(
                        kt_sb[m][:, t0:t0 + TCH], pk, bk_sb[m])
                return f, 1750.0

            def u_pv(ti, s):
                def f():
                    st = store[ti]
                    t0 = ti * TCH
                    pv = psm_tile([128, HG], f32, "pv")
                    for c in range(NCC):
                        mm(pv, st["xt"][c][:, s * 128:(s + 1) * 128],
                           wv_sb[c], c == 0, c == NCC - 1)
                    vt = v_sb[(t0 + s * 128) // 128]
                    nc.vector.tensor_add(
                        vt.rearrange("p (h e) -> p h e", h=NHL)[:, :, 0:Dh],
                        pv.rearrange("p (h d) -> p h d", h=NHL),
                        bv_bc.rearrange("p (h d) -> p h d", h=NHL))
                return f, 1750.0

            def u_qk(ti, hp, g):
                h0, h1 = 2 * hp, 2 * hp + 1

                def f():
                    st = store[ti]
                    qt = st["qt"]
                    pair = store.setdefault(("po", ti, hp), {})
                    if g == 0:
                        pair["po0"] = psO.tile([128, TCH], f32, tag="o", name="po0")
                        pair["po1"] = psO.tile([128, TCH], f32, tag="o", name="po1")
                    qt0 = qt[hp][0:64, :]
                    qt1 = qt[hp][64:128, :]
                    ps0 = psS.tile([128, 2 * TCH], f32, tag="sT", name="ps0")
                    ps1 = psS.tile([128, 2 * TCH], f32, tag="sT", name="ps1")
                    # --- QK^T (+ triangular -1e9 for diagonal k-tiles) ---
                    for half, kk in enumerate((2 * g, 2 * g + 1)):
                        j = kk - 4 * ti  # >=0: diagonal k-tile index
                        q0 = 128 * j if j >= 0 else 0
                        c0 = half * TCH
                        diag = j >= 0
                        mm(ps0[:, c0 + q0:c0 + TCH],
                           kt_sb[hp][0:64, kk * 128:(kk + 1) * 128],
                           qt0[:, q0:TCH], True, not diag,
                           skip_group_check=True)
                        mm(ps1[:, c0 + q0:c0 + TCH],
                           kt_sb[hp][64:128, kk * 128:(kk + 1) * 128],
                           qt1[:, q0:TCH], True, not diag,
                           skip_group_check=True)
                        if diag:
                            mm(ps0[:, c0 + q0:c0 + q0 + 128], ident, trin,
                               False, True, skip_group_check=True)
                            mm(ps1[:, c0 + q0:c0 + q0 + 128], ident, trin,
                               False, True, skip_group_check=True)
                    # --- exp (scalar engine) ---
                    es0 = esp.tile([128, 2 * TCH], f32r, tag="es", name="es0")
                    es1 = esp.tile([128, 2 * TCH], f32r, tag="es", name="es1")
                    nc.scalar.activation(es0, ps0, AF.Exp, scale=0.125)
                    nc.scalar.activation(es1, ps1, AF.Exp, scale=0.125)
                    pair[("es", g)] = (es0, es1)

                w = sum(TCH - (max(kk - 4 * ti, 0) * 128)
                        for kk in (2 * g, 2 * g + 1))
                pe = w / 2.4 + (420.0 if 2 * g + 1 >= 4 * ti else 0.0)
                return f, pe

            def u_pvat(ti, hp, g):
                nkt = 4 * (ti + 1)
                h0, h1 = 2 * hp, 2 * hp + 1

                def f():
                    pair = store[("po", ti, hp)]
                    po0, po1 = pair["po0"], pair["po1"]
                    es0, es1 = pair.pop(("es", g))
                    for half, kk in enumerate((2 * g, 2 * g + 1)):
                        j = kk - 4 * ti
                        q0 = 128 * j if j >= 0 else 0
                        c0 = half * TCH
                        first = kk == 0
                        last = kk == nkt - 1
                        mm(po0[0:Dh + 1, q0:TCH],
                           v_sb[kk][:, h0 * 65:(h0 + 1) * 65],
                           es0[:, c0 + q0:c0 + TCH], first, last,
                           skip_group_check=True)
                        mm(po1[0:Dh + 1, q0:TCH],
                           v_sb[kk][:, h1 * 65:(h1 + 1) * 65],
                           es1[:, c0 + q0:c0 + TCH], first, last,
                           skip_group_check=True)

                w = sum(TCH - (max(kk - 4 * ti, 0) * 128)
                        for kk in (2 * g, 2 * g + 1))
                return f, w / 2.4 * 2

            def u_norm(ti, hp):
                def f():
                    pair = store[("po", ti, hp)]
                    po0, po1 = pair["po0"], pair["po1"]
                    dsb0 = rp.tile([1, TCH], f32r, tag="rd0", name="dsb0")
                    dsb1 = rp.tile([1, TCH], f32r, tag="rd1", name="dsb1")
                    nc.vector.tensor_copy(dsb0, po0[Dh:Dh + 1, :])
                    nc.vector.tensor_copy(dsb1, po1[Dh:Dh + 1, :])
                    rb0 = psm_tile([128, TCH], f32, "rb0")
                    mm(rb0[0:64, :], ones, dsb0, True, True)
                    rb1 = psm_tile([128, TCH], f32, "rb1")
                    mm(rb1[0:64, :], ones, dsb1, True, True)
                    rbs0 = rp.tile([64, TCH], f32, tag="rbs0", name="rbs0")
                    rbs1 = rp.tile([64, TCH], f32, tag="rbs1", name="rbs1")
                    nc.vector.reciprocal_approx_fast(rbs0, rb0[0:64, :])
                    nc.vector.reciprocal_approx_fast(rbs1, rb1[0:64, :])
                    ot = ot_store[(ti, hp)] = otp.tile(
                        [128, TCH], bf16, name=f"ot{hp}", tag=f"ot{hp}")
                    nc.vector.tensor_mul(ot[0:64, :], po0[0:Dh, :], rbs0)
                    nc.vector.tensor_mul(ot[64:128, :], po1[0:Dh, :], rbs1)
                return f, 750.0

            def u_oproj(ti, s, n):
                def f():
                    py = psm_tile([128, TCH], f32, "py")
                    for m in range(NMT):
                        mm(py, ot_store[(ti, m)][:, s * 128:(s + 1) * 128],
                           wo_sb[m][:, n * TCH:(n + 1) * TCH],
                           m == 0, m == NMT - 1)
                    yt = yst.tile([128, TCH], bf16, tag="yst", name="yt")
                    nc.vector.tensor_copy(yt, py)
                    nc.sync.dma_start(
                        out=y_d[ti * TCH + s * 128:ti * TCH + (s + 1) * 128,
                                n * TCH:(n + 1) * TCH],
                        in_=yt)
                return f, 900.0

            # ======== schedule: paced emission ========
            ACT_GRP = 2800.0

            def proj_units(ti, parts=("ldxt", "pq", "pk", "pv")):
                us = []
                if "ldxt" in parts:
                    us += [u_ldxt(ti, c) for c in range(NCC)]
                if "pq" in parts:
                    us += [u_pq(ti, m) for m in range(NMT)]
                if "pk" in parts:
                    us += [u_pk(ti, m) for m in range(NMT)]
                if "pv" in parts:
                    us += [u_pv(ti, s) for s in range(4)]
                return us

            def oproj_units(ti):
                return [u_oproj(ti, s, n) for s in range(4) for n in range(2)]

            # filler entries: (fn, pe_cost, deadline) where deadline is
            # (chunk, 0) = before that chunk's attention, (3, 1) = before
            # chunk 3's diagonal grps, (9, 0) = anytime.
            filler = deque()
            D = {"v": 0.0}

            def flush(deadline):
                keep = deque()
                while filler:
                    fn, pe, dl = filler.popleft()
                    if dl <= deadline:
                        fn()
                    else:
                        keep.append((fn, pe, dl))
                filler.extend(keep)
                D["v"] = 0.0

            # proj(0): emit only what pair (0,hp0) needs inline; the rest
            # paces into chunk-0's attention with per-pair deadlines
            for u in proj_units(0, parts=("ldxt",)):
                u[0]()
            for m in (0,):
                u_pq(0, m)[0]()
                u_pk(0, m)[0]()
            for s in range(4):
                u_pv(0, s)[0]()
            for m in (1, 2, 3):
                filler.append((u_pq(0, m)[0], 1750.0, (0, m)))
                filler.append((u_pk(0, m)[0], 1750.0, (0, m)))

            for ti in range(NTC):
                if ti == 0:
                    for u in proj_units(1):
                        filler.append((u[0], u[1], (1, 0)))
                elif ti == 1:
                    for u in proj_units(2):
                        filler.append((u[0], u[1], (2, 0)))
                    for u in oproj_units(0):
                        filler.append((u[0], u[1], (2, 0)))
                elif ti == 2:
                    for u in proj_units(3, parts=("ldxt", "pq")):
                        filler.append((u[0], u[1], (3, 0)))
                    for u in proj_units(3, parts=("pk", "pv")):
                        filler.append((u[0], u[1], (3, 1)))
                    for u in oproj_units(1):
                        filler.append((u[0], u[1], (3, 1)))
                else:
                    for u in oproj_units(2):
                        filler.append((u[0], u[1], (9, 0)))
                flush((ti, 0))

                ngrp = 2 * (ti + 1)
                for hp in range(NMT):
                    if ti == 0 and hp > 0:
                        flush((0, hp))
                    for g in range(ngrp + 1):
                        if ti == 3 and 2 * g + 1 >= 4 * ti:
                            flush((3, 1))
                        if g < ngrp:
                            fn, pe = u_qk(ti, hp, g)
                            fn()
                            D["v"] += ACT_GRP - pe
                        while D["v"] > 0 and filler:
                            ffn, fpe, _ = filler.popleft()
                            ffn()
                            D["v"] -= fpe
                        if not filler:
                            D["v"] = min(D["v"], 1.5 * ACT_GRP)
                        if g > 0:
                            fn, pe = u_pvat(ti, hp, g - 1)
                            fn()
                            D["v"] -= pe
                    fn, pe = u_norm(ti, hp)
                    fn()
                    D["v"] -= pe

            flush((9, 0))
            for fn, pe in oproj_units(3):
                fn()
    nc.compile()
    return nc


def _get_nc():
    if "nc" not in _CACHE:
        _CACHE["nc"] = _build_nc()
    return _CACHE["nc"]


def _in_maps(x, w_qkv, b_qkv, w_out):
    import ml_dtypes
    bf = ml_dtypes.bfloat16
    p = np.arange(128)
    trin = np.where(p[None, :] < p[:, None], -1e9, 0.0).astype(bf)
    ident = np.eye(128, dtype=np.float32).astype(bf)
    maps = []
    for c in range(8):
        b, g = c // 2, c % 2
        s = g * HG
        maps.append({
            "x": np.ascontiguousarray(x[b].T).astype(bf),
            "wq": np.ascontiguousarray(w_qkv[:, s:s + HG]).astype(bf),
            "wk": np.ascontiguousarray(w_qkv[:, C + s:C + s + HG]).astype(bf),
            "wv": np.ascontiguousarray(w_qkv[:, 2 * C + s:2 * C + s + HG]).astype(bf),
            "bq": np.ascontiguousarray(b_qkv[s:s + HG]),
            "bk": np.ascontiguousarray(b_qkv[C + s:C + s + HG]),
            "bv": np.ascontiguousarray(b_qkv[2 * C + s:2 * C + s + HG]),
            "wo": np.ascontiguousarray(w_out[s:s + HG, :]).astype(bf),
            "ident": ident,
            "trin": trin,
            "ones": np.ones((1, 64), dtype=np.float32),
            "vones": np.ones((128, NHL), dtype=np.float32),
        })
    return maps


def _run(x, w_qkv, b_qkv, w_out, b_out, trace=False, tmpdir=None):
    from concourse import bass_utils
    nc = _get_nc()
    maps = _in_maps(x, w_qkv, b_qkv, w_out)
    # the device occasionally reports a transient unrecoverable-exec error
    # right after a reset; one retry clears it
    last = None
    for attempt in range(3):
        try:
            res = bass_utils.run_bass_kernel_spmd(
                nc, maps, core_ids=list(range(8)), trace=trace, tmpdir=tmpdir)
            break
        except Exception as e:
            last = e
            if attempt == 2:
                raise
    else:
        raise last
    ys = [np.asarray(res.results[c]["y"]).astype(np.float32) for c in range(8)]
    out = np.stack([ys[2 * b] + ys[2 * b + 1] for b in range(B)])
    out += np.asarray(b_out, dtype=np.float32)[None, None, :]
    return out.astype(np.float32), res


def kernel(x, w_qkv, b_qkv, w_out, b_out):
    x = np.asarray(x, dtype=np.float32)
    w_qkv = np.asarray(w_qkv, dtype=np.float32)
    b_qkv = np.asarray(b_qkv, dtype=np.float32)
    w_out = np.asarray(w_out, dtype=np.float32)
    b_out = np.asarray(b_out, dtype=np.float32)
    out, _ = _run(x, w_qkv, b_qkv, w_out, b_out, trace=False)
    return out


# revision 22
# speedup vs baseline: 1.0067x; 1.0067x over previous
"""Causal self-attention on 8 trn2 NeuronCores.

Sharding: core c = (batch b = c//2, head-group g = c%2). Each core computes
QKV projection for its 8 heads of its batch, causal flash-attention in a
transposed (S^T) layout, and a partial out-projection (its 512 rows of
w_out). Host sums the two partials per batch and adds b_out.

v2 design (from HW microbenchmarks):
- bf16 operands for projections / QK^T / out-projection (same PE rate as
  f32r but half the DMA, half the LDWEIGHTS time, half the DVE cost on
  PSUM->SBUF moves). PV stays f32r: its M=65 output (64 dims + ones column
  for the softmax denominator) is rejected by codegen for 16-bit dtypes.
- The causal mask runs on the PE: for each diagonal k-tile the QK^T matmul
  is column-restricted to q >= tile base, and one extra N=128 matmul
  accumulates -1e9 over the triangular block (ident x trin), so exp gives
  exact zeros and the DVE mask multiplies are gone.
- Normalization: reciprocal of the denominator row straight from PSUM,
  PE-broadcast via a ones[1,64] stationary, one SBUF bounce, two muls.
- Scheduler: attention is emitted grp-by-grp (grp = head-pair x 2 k-tiles);
  after each grp, filler units (next chunk's projection, previous chunks'
  out-projection) are emitted to fill the PE while the scalar engine's exp
  (the per-grp bottleneck: ~2.3us vs ~1.3us PE) catches up. The PE never
  idles a HAM window, so the clock stays at 2.4 GHz. Chunk 3 keeps its own
  k/v projection and two chunks' out-projections as late filler.
"""

import numpy as np

B = 4
T = 2048
C = 1024
HG = 512          # head channels per core (8 heads x 64)
Dh = 64
NHL = 8           # local heads per core
TCH = 512         # T-chunk (q-chunk) width
NTC = T // TCH    # 4
NCC = C // 128    # 8 contraction chunks for projections
NMT = HG // 128   # 4 row-tiles of q/k channels
VW = NHL * (Dh + 1)   # 520: V tiles with a ones column per head

_CACHE = {}


def _build_nc():
    import concourse.bass as bass
    import concourse.bacc as bacc
    import concourse.tile as tile
    import concourse.mybir as mybir
    from collections import deque

    f32 = mybir.dt.float32
    f32r = mybir.dt.float32r
    bf16 = mybir.dt.bfloat16
    AF = mybir.ActivationFunctionType

    nc = bacc.Bacc("TRN2", target_bir_lowering=False, debug=False,
                   enable_asserts=False)
    x_d = nc.dram_tensor("x", [C, T], bf16, kind="ExternalInput").ap()
    wq_d = nc.dram_tensor("wq", [C, HG], bf16, kind="ExternalInput").ap()
    wk_d = nc.dram_tensor("wk", [C, HG], bf16, kind="ExternalInput").ap()
    wv_d = nc.dram_tensor("wv", [C, HG], bf16, kind="ExternalInput").ap()
    bq_d = nc.dram_tensor("bq", [HG], f32, kind="ExternalInput").ap()
    bk_d = nc.dram_tensor("bk", [HG], f32, kind="ExternalInput").ap()
    bv_d = nc.dram_tensor("bv", [HG], f32, kind="ExternalInput").ap()
    wo_d = nc.dram_tensor("wo", [HG, C], bf16, kind="ExternalInput").ap()
    id_d = nc.dram_tensor("ident", [128, 128], bf16, kind="ExternalInput").ap()
    mk_d = nc.dram_tensor("mask", [2, 128, 2 * TCH], bf16,
                          kind="ExternalInput").ap()
    on_d = nc.dram_tensor("ones", [1, 64], f32r, kind="ExternalInput").ap()
    vo_d = nc.dram_tensor("vones", [128, NHL], f32r, kind="ExternalInput").ap()
    y_d = nc.dram_tensor("y", [T, C], bf16, kind="ExternalOutput").ap()

    def mm(out, lhsT, rhs, start, stop, **kw):
        nc.tensor.matmul(out, lhsT, rhs, start=start, stop=stop, **kw)

    with tile.TileContext(nc) as tc:
        with tc.tile_pool(name="wp", bufs=1) as wp, \
             tc.tile_pool(name="ktp", bufs=1) as ktp, \
             tc.tile_pool(name="vp", bufs=1) as vp, \
             tc.tile_pool(name="qtp", bufs=2) as qtp, \
             tc.tile_pool(name="xtp", bufs=2) as xtp, \
             tc.tile_pool(name="esp", bufs=6) as esp, \
             tc.tile_pool(name="rp", bufs=1) as rp, \
             tc.tile_pool(name="otp", bufs=3) as otp, \
             tc.tile_pool(name="yst", bufs=2) as yst, \
             tc.tile_pool(name="psM", bufs=2, space="PSUM") as psM, \
             tc.tile_pool(name="psS", bufs=2, space="PSUM") as psS, \
             tc.tile_pool(name="psO", bufs=2, space="PSUM") as psO:

            # ---- constants (sync/HWDGE: needed first) ----
            ident = wp.tile([128, 128], bf16, tag="ident")
            nc.sync.dma_start(out=ident, in_=id_d)
            masks = [wp.tile([128, 2 * TCH], bf16, name=f"mk{j}", tag=f"mk{j}")
                     for j in range(2)]
            for j in range(2):
                nc.sync.dma_start(out=masks[j], in_=mk_d[j])
            ones = wp.tile([1, 64], f32r, tag="ones")
            nc.sync.dma_start(out=ones, in_=on_d)
            # (chunk-0 x^T tiles are loaded by the first ldxt units below)

            # ---- weights etc on the gpsimd/SWDGE queues so the x-chunk
            # loads (sync/HWDGE) aren't queued behind the weights ----
            wq_sb = [wp.tile([128, HG], bf16, name=f"wq{c}", tag=f"wq{c}")
                     for c in range(NCC)]
            wk_sb = [wp.tile([128, HG], bf16, name=f"wk{c}", tag=f"wk{c}")
                     for c in range(NCC)]
            wv_sb = [wp.tile([128, HG], bf16, name=f"wv{c}", tag=f"wv{c}")
                     for c in range(NCC)]
            for c in range(NCC):
                nc.gpsimd.dma_start(out=wq_sb[c], in_=wq_d[c * 128:(c + 1) * 128, :])
            for c in range(NCC):
                nc.gpsimd.dma_start(out=wk_sb[c], in_=wk_d[c * 128:(c + 1) * 128, :])
            for c in range(NCC):
                nc.gpsimd.dma_start(out=wv_sb[c], in_=wv_d[c * 128:(c + 1) * 128, :])
            wo_sb = [wp.tile([128, C], bf16, name=f"wo{m}", tag=f"wo{m}")
                     for m in range(NMT)]
            bq_sb = [wp.tile([128, 1], f32, name=f"bq{m}", tag=f"bq{m}")
                     for m in range(NMT)]
            bk_sb = [wp.tile([128, 1], f32, name=f"bk{m}", tag=f"bk{m}")
                     for m in range(NMT)]
            for m in range(NMT):
                nc.gpsimd.dma_start(
                    out=bq_sb[m],
                    in_=bq_d[m * 128:(m + 1) * 128].rearrange("(p o) -> p o", o=1))
                nc.gpsimd.dma_start(
                    out=bk_sb[m],
                    in_=bk_d[m * 128:(m + 1) * 128].rearrange("(p o) -> p o", o=1))
            bv_bc = wp.tile([128, HG], f32, tag="bvbc")
            bv_src = bass.AP(tensor=bv_d.tensor, offset=bv_d.offset,
                             ap=[[0, 128]] + list(bv_d.ap))
            nc.gpsimd.dma_start(out=bv_bc, in_=bv_src)
            for m in range(NMT):
                nc.gpsimd.dma_start(out=wo_sb[m], in_=wo_d[m * 128:(m + 1) * 128, :])

            kt_sb = [ktp.tile([128, T], bf16, name=f"kt{m}", tag=f"kt{m}")
                     for m in range(NMT)]
            v_sb = [vp.tile([128, VW], f32r, name=f"v{t}", tag=f"v{t}")
                    for t in range(T // 128)]
            for t in range(T // 128):
                nc.gpsimd.dma_start(
                    out=v_sb[t].rearrange("p (h e) -> p h e", h=NHL)[:, :, Dh:Dh + 1],
                    in_=vo_d.rearrange("p (h e) -> p h e", e=1))

            # ======== unit closures ========
            store = {}
            ot_store = {}
            def psm_tile(shape, dtype, name):
                return psM.tile(shape, dtype, tag="mm", name=name)

            def u_ldxt(ti, c):
                def f():
                    t0 = ti * TCH
                    if ti not in store:
                        store[ti] = {"xt": [None] * NCC, "qt": [None] * NMT}
                    st = store[ti]
                    xt = xtp.tile([128, TCH], bf16, name=f"xt{c}", tag=f"xt{c}")
                    st["xt"][c] = xt
                    nc.sync.dma_start(
                        out=xt, in_=x_d[c * 128:(c + 1) * 128, t0:t0 + TCH])
                return f, 0.0

            def u_pq(ti, m):
                def f():
                    st = store[ti]
                    pq = psm_tile([128, TCH], f32, "pq")
                    for c in range(NCC):
                        mm(pq, wq_sb[c][:, m * 128:(m + 1) * 128],
                           st["xt"][c], c == 0, c == NCC - 1)
                    qtm = qtp.tile([128, TCH], bf16, name=f"qt{m}", tag=f"qt{m}")
                    st["qt"][m] = qtm
                    nc.vector.tensor_scalar_add(qtm, pq, bq_sb[m])
                return f, 1750.0

            def u_pk(ti, m):
                def f():
                    st = store[ti]
                    t0 = ti * TCH
                    pk = psm_tile([128, TCH], f32, "pk")
                    for c in range(NCC):
                        mm(pk, wk_sb[c][:, m * 128:(m + 1) * 128],
                           st["xt"][c], c == 0, c == NCC - 1)
                    nc.vector.tensor_scalar_add(
                        kt_sb[m][:, t0:t0 + TCH], pk, bk_sb[m])
                return f, 1750.0

            def u_pv(ti, s):
                def f():
                    st = store[ti]
                    t0 = ti * TCH
                    pv = psm_tile([128, HG], f32, "pv")
                    for c in range(NCC):
                        mm(pv, st["xt"][c][:, s * 128:(s + 1) * 128],
                           wv_sb[c], c == 0, c == NCC - 1)
                    vt = v_sb[(t0 + s * 128) // 128]
                    nc.vector.tensor_add(
                        vt.rearrange("p (h e) -> p h e", h=NHL)[:, :, 0:Dh],
                        pv.rearrange("p (h d) -> p h d", h=NHL),
                        bv_bc.rearrange("p (h d) -> p h d", h=NHL))
                return f, 1750.0

            def u_qk(ti, hp, g):
                h0, h1 = 2 * hp, 2 * hp + 1

                def f():
                    st = store[ti]
                    qt = st["qt"]
                    pair = store.setdefault(("po", ti, hp), {})
                    if g == 0:
                        pair["po0"] = psO.tile([128, TCH], f32, tag="o", name="po0")
                        pair["po1"] = psO.tile([128, TCH], f32, tag="o", name="po1")
                    qt0 = qt[hp][0:64, :]
                    qt1 = qt[hp][64:128, :]
                    ps0 = psS.tile([128, 2 * TCH], f32, tag="sT", name="ps0")
                    ps1 = psS.tile([128, 2 * TCH], f32, tag="sT", name="ps1")
                    # --- QK^T (+ triangular -1e9 for diagonal k-tiles) ---
                    for half, kk in enumerate((2 * g, 2 * g + 1)):
                        j = kk - 4 * ti  # >=0: diagonal k-tile index
                        q0 = 128 * j if j >= 0 else 0
                        c0 = half * TCH
                        diag = j >= 0
                        mm(ps0[:, c0 + q0:c0 + TCH],
                           kt_sb[hp][0:64, kk * 128:(kk + 1) * 128],
                           qt0[:, q0:TCH], True, True,
                           skip_group_check=True)
                        mm(ps1[:, c0 + q0:c0 + TCH],
                           kt_sb[hp][64:128, kk * 128:(kk + 1) * 128],
                           qt1[:, q0:TCH], True, True,
                           skip_group_check=True)
                    # --- exp (scalar engine) ---
                    es0 = esp.tile([128, 2 * TCH], f32r, tag="es", name="es0")
                    es1 = esp.tile([128, 2 * TCH], f32r, tag="es", name="es1")
                    nc.scalar.activation(es0, ps0, AF.Exp, scale=0.125)
                    nc.scalar.activation(es1, ps1, AF.Exp, scale=0.125)
                    ngrp = 2 * (ti + 1)
                    dj = g - (ngrp - 2)
                    if dj >= 0:
                        nc.vector.tensor_mul(es0, es0, masks[dj])
                        nc.vector.tensor_mul(es1, es1, masks[dj])
                    pair[("es", g)] = (es0, es1)

                w = sum(TCH - (max(kk - 4 * ti, 0) * 128)
                        for kk in (2 * g, 2 * g + 1))
                return f, w / 2.4

            def u_pvat(ti, hp, g):
                nkt = 4 * (ti + 1)
                h0, h1 = 2 * hp, 2 * hp + 1

                def f():
                    pair = store[("po", ti, hp)]
                    po0, po1 = pair["po0"], pair["po1"]
                    es0, es1 = pair.pop(("es", g))
                    for half, kk in enumerate((2 * g, 2 * g + 1)):
                        j = kk - 4 * ti
                        q0 = 128 * j if j >= 0 else 0
                        c0 = half * TCH
                        first = kk == 0
                        last = kk == nkt - 1
                        mm(po0[0:Dh + 1, q0:TCH],
                           v_sb[kk][:, h0 * 65:(h0 + 1) * 65],
                           es0[:, c0 + q0:c0 + TCH], first, last,
                           skip_group_check=True)
                        mm(po1[0:Dh + 1, q0:TCH],
                           v_sb[kk][:, h1 * 65:(h1 + 1) * 65],
                           es1[:, c0 + q0:c0 + TCH], first, last,
                           skip_group_check=True)

                w = sum(TCH - (max(kk - 4 * ti, 0) * 128)
                        for kk in (2 * g, 2 * g + 1))
                return f, w / 2.4 * 2

            def u_norm(ti, hp):
                def f():
                    pair = store[("po", ti, hp)]
                    po0, po1 = pair["po0"], pair["po1"]
                    dsb0 = rp.tile([1, TCH], f32r, tag="rd0", name="dsb0")
                    dsb1 = rp.tile([1, TCH], f32r, tag="rd1", name="dsb1")
                    nc.vector.tensor_copy(dsb0, po0[Dh:Dh + 1, :])
                    nc.vector.tensor_copy(dsb1, po1[Dh:Dh + 1, :])
                    rb0 = psm_tile([128, TCH], f32, "rb0")
                    mm(rb0[0:64, :], ones, dsb0, True, True)
                    rb1 = psm_tile([128, TCH], f32, "rb1")
                    mm(rb1[0:64, :], ones, dsb1, True, True)
                    rbs0 = rp.tile([64, TCH], f32, tag="rbs0", name="rbs0")
                    rbs1 = rp.tile([64, TCH], f32, tag="rbs1", name="rbs1")
                    nc.vector.reciprocal_approx_fast(rbs0, rb0[0:64, :])
                    nc.vector.reciprocal_approx_fast(rbs1, rb1[0:64, :])
                    ot = ot_store[(ti, hp)] = otp.tile(
                        [128, TCH], bf16, name=f"ot{hp}", tag=f"ot{hp}")
                    nc.vector.tensor_mul(ot[0:64, :], po0[0:Dh, :], rbs0)
                    nc.vector.tensor_mul(ot[64:128, :], po1[0:Dh, :], rbs1)
                return f, 750.0

            def u_oproj(ti, s, n):
                def f():
                    py = psm_tile([128, TCH], f32, "py")
                    for m in range(NMT):
                        mm(py, ot_store[(ti, m)][:, s * 128:(s + 1) * 128],
                           wo_sb[m][:, n * TCH:(n + 1) * TCH],
                           m == 0, m == NMT - 1)
                    yt = yst.tile([128, TCH], bf16, tag="yst", name="yt")
                    nc.vector.tensor_copy(yt, py)
                    nc.sync.dma_start(
                        out=y_d[ti * TCH + s * 128:ti * TCH + (s + 1) * 128,
                                n * TCH:(n + 1) * TCH],
                        in_=yt)
                return f, 900.0

            # ======== schedule: paced emission ========
            ACT_GRP = 2800.0

            def proj_units(ti, parts=("ldxt", "pq", "pk", "pv")):
                us = []
                if "ldxt" in parts:
                    us += [u_ldxt(ti, c) for c in range(NCC)]
                if "pq" in parts:
                    us += [u_pq(ti, m) for m in range(NMT)]
                if "pk" in parts:
                    us += [u_pk(ti, m) for m in range(NMT)]
                if "pv" in parts:
                    us += [u_pv(ti, s) for s in range(4)]
                return us

            def oproj_units(ti):
                return [u_oproj(ti, s, n) for s in range(4) for n in range(2)]

            # filler entries: (fn, pe_cost, deadline) where deadline is
            # (chunk, 0) = before that chunk's attention, (3, 1) = before
            # chunk 3's diagonal grps, (9, 0) = anytime.
            filler = deque()
            D = {"v": 0.0}

            def flush(deadline):
                keep = deque()
                while filler:
                    fn, pe, dl = filler.popleft()
                    if dl <= deadline:
                        fn()
                    else:
                        keep.append((fn, pe, dl))
                filler.extend(keep)
                D["v"] = 0.0

            # proj(0): emit only what pair (0,hp0) needs inline; the rest
            # paces into chunk-0's attention with per-pair deadlines
            for u in proj_units(0, parts=("ldxt",)):
                u[0]()
            for m in (0,):
                u_pq(0, m)[0]()
                u_pk(0, m)[0]()
            for s in range(4):
                u_pv(0, s)[0]()
            for m in (1, 2, 3):
                filler.append((u_pq(0, m)[0], 1750.0, (0, m)))
                filler.append((u_pk(0, m)[0], 1750.0, (0, m)))

            for ti in range(NTC):
                if ti == 0:
                    for u in proj_units(1):
                        filler.append((u[0], u[1], (1, 0)))
                elif ti == 1:
                    for u in proj_units(2):
                        filler.append((u[0], u[1], (2, 0)))
                    for u in oproj_units(0):
                        filler.append((u[0], u[1], (2, 0)))
                elif ti == 2:
                    for u in proj_units(3, parts=("ldxt", "pq")):
                        filler.append((u[0], u[1], (3, 0)))
                    for u in proj_units(3, parts=("pk", "pv")):
                        filler.append((u[0], u[1], (3, 1)))
                    for u in oproj_units(1):
                        filler.append((u[0], u[1], (3, 1)))
                else:
                    for u in oproj_units(2):
                        filler.append((u[0], u[1], (9, 0)))
                flush((ti, 0))

                ngrp = 2 * (ti + 1)
                for hp in range(NMT):
                    if ti == 0 and hp > 0:
                        flush((0, hp))
                    for g in range(ngrp + 1):
                        if ti == 3 and 2 * g + 1 >= 4 * ti:
                            flush((3, 1))
                        if g < ngrp:
                            fn, pe = u_qk(ti, hp, g)
                            fn()
                            D["v"] += ACT_GRP - pe
                        while D["v"] > -ACT_GRP and filler:
                            ffn, fpe, _ = filler.popleft()
                            ffn()
                            D["v"] -= fpe
                        if not filler:
                            D["v"] = min(D["v"], 1.5 * ACT_GRP)
                        if g > 0:
                            fn, pe = u_pvat(ti, hp, g - 1)
                            fn()
                            D["v"] -= pe
                    fn, pe = u_norm(ti, hp)
                    fn()
                    D["v"] -= pe

            flush((9, 0))
            for fn, pe in oproj_units(3):
                fn()
    nc.compile()
    return nc


def _get_nc():
    if "nc" not in _CACHE:
        _CACHE["nc"] = _build_nc()
    return _CACHE["nc"]


def _in_maps(x, w_qkv, b_qkv, w_out):
    import ml_dtypes
    bf = ml_dtypes.bfloat16
    p = np.arange(128)[:, None]
    f = np.arange(TCH)[None, :]
    quads = [(f >= j * 128 + p).astype(np.float32) for j in range(4)]
    masks = np.stack([np.concatenate([quads[0], quads[1]], axis=1),
                      np.concatenate([quads[2], quads[3]], axis=1)]).astype(bf)
    ident = np.eye(128, dtype=np.float32).astype(bf)
    maps = []
    for c in range(8):
        b, g = c // 2, c % 2
        s = g * HG
        maps.append({
            "x": np.ascontiguousarray(x[b].T).astype(bf),
            "wq": np.ascontiguousarray(w_qkv[:, s:s + HG]).astype(bf),
            "wk": np.ascontiguousarray(w_qkv[:, C + s:C + s + HG]).astype(bf),
            "wv": np.ascontiguousarray(w_qkv[:, 2 * C + s:2 * C + s + HG]).astype(bf),
            "bq": np.ascontiguousarray(b_qkv[s:s + HG]),
            "bk": np.ascontiguousarray(b_qkv[C + s:C + s + HG]),
            "bv": np.ascontiguousarray(b_qkv[2 * C + s:2 * C + s + HG]),
            "wo": np.ascontiguousarray(w_out[s:s + HG, :]).astype(bf),
            "ident": ident,
            "mask": masks,
            "ones": np.ones((1, 64), dtype=np.float32),
            "vones": np.ones((128, NHL), dtype=np.float32),
        })
    return maps


def _run(x, w_qkv, b_qkv, w_out, b_out, trace=False, tmpdir=None):
    from concourse import bass_utils
    nc = _get_nc()
    maps = _in_maps(x, w_qkv, b_qkv, w_out)
    # the device occasionally reports a transient unrecoverable-exec error
    # right after a reset; one retry clears it
    last = None
    for attempt in range(3):
        try:
            res = bass_utils.run_bass_kernel_spmd(
                nc, maps, core_ids=list(range(8)), trace=trace, tmpdir=tmpdir)
            break
        except Exception as e:
            last = e
            if attempt == 2:
                raise
    else:
        raise last
    ys = [np.asarray(res.results[c]["y"]).astype(np.float32) for c in range(8)]
    out = np.stack([ys[2 * b] + ys[2 * b + 1] for b in range(B)])
    out += np.asarray(b_out, dtype=np.float32)[None, None, :]
    return out.astype(np.float32), res


def kernel(x, w_qkv, b_qkv, w_out, b_out):
    x = np.asarray(x, dtype=np.float32)
    w_qkv = np.asarray(w_qkv, dtype=np.float32)
    b_qkv = np.asarray(b_qkv, dtype=np.float32)
    w_out = np.asarray(w_out, dtype=np.float32)
    b_out = np.asarray(b_out, dtype=np.float32)
    out, _ = _run(x, w_qkv, b_qkv, w_out, b_out, trace=False)
    return out
